# revision 1
# baseline (speedup 1.0000x reference)
"""LBN layer kernel for Trainium2 (8 NeuronCores, pure data parallel).

Inputs: E, px, py, pz each [262144, 16] f32.
Output: [262144, 424] = concat(E, px, py, pz, cross_z, cross_x, cross_y)
where cross(a,b)[r, (i,j)] = a[r,i]*b[r,j] - a[r,j]*b[r,i] over the 120
upper-triangle pairs (i<j), for (a,b) in [(px,py),(py,pz),(pz,px)].

Per-core scheme (32768 rows/core):
- Row r_local = p*256 + s maps to SBUF partition p, stripe slot s, so both
  the load and store DMAs are fully contiguous per partition.
- Per tile (nb row-blocks per partition): the 4 inputs are DMA'd straight
  into the out tile (cols 0:64 of each 424-wide block); for each feature,
  an outer-product M[b,i,j] = a_i*b_j (one wide tensor_tensor) followed by
  15 compact subtracts cross(i,j) = M[i,j] - M[j,i] written directly into
  the triu-packed output columns.
"""

import os

import numpy as np

import concourse.bass as bass
import concourse.tile as tile
from concourse import bacc, bass_utils, mybir

F32 = mybir.dt.float32
N = 16
NPAIR = (N * (N - 1)) // 2  # 120
OUTW = 4 * N + 3 * NPAIR  # 424
TOTAL_ROWS = 262144
NCORES = 8
ROWS_PER_CORE = TOTAL_ROWS // NCORES  # 32768

FEAT = [("px", "py"), ("py", "pz"), ("pz", "px")]
COL = {"E": 0, "px": 16, "py": 32, "pz": 48}


def build_kernel(
    nrows=ROWS_PER_CORE,
    nb=16,
    mul_eng=("v", "v", "g"),
    sub_eng=("v", "g", "v"),
    obufs=4,
    mbufs=3,
):
    """Build the per-core Bass module. nb = row-blocks per partition per tile."""
    stripe = nrows // 128
    assert stripe % nb == 0
    nt = stripe // nb

    nc = bacc.Bacc("TRN2", target_bir_lowering=False, debug=False)
    ins = {
        k: nc.dram_tensor(k, [nrows, N], F32, kind="ExternalInput").ap()
        for k in ["E", "px", "py", "pz"]
    }
    out = nc.dram_tensor("out", [nrows, OUTW], F32, kind="ExternalOutput").ap()
    inv = {k: v.rearrange("(p s) f -> p s f", p=128) for k, v in ins.items()}
    outv = out.rearrange("(p s) f -> p s f", p=128)

    def eng(c):
        return {"v": nc.vector, "g": nc.gpsimd}[c]

    with tile.TileContext(nc) as tc:
        with (
            tc.tile_pool(name="ot", bufs=obufs) as opool,
            tc.tile_pool(name="mt", bufs=mbufs) as mpool,
        ):
            for t in range(nt):
                ot = opool.tile([128, OUTW * nb], F32)
                ov = ot[:].rearrange("p (b c) -> p b c", c=OUTW)
                for k in ["E", "px", "py", "pz"]:
                    nc.sync.dma_start(
                        out=ov[:, :, COL[k] : COL[k] + N],
                        in_=inv[k][:, t * nb : (t + 1) * nb, :],
                    )
                off0 = 4 * N
                for f, (an, bn) in enumerate(FEAT):
                    mt = mpool.tile([128, N * N * nb], F32)
                    mv = mt[:].rearrange("p (b c) -> p b c", c=N * N)
                    mm = mv.rearrange("p b (i j) -> p b i j", j=N)
                    a = ov[:, :, COL[an] : COL[an] + N]
                    b = ov[:, :, COL[bn] : COL[bn] + N]
                    aap = a.unsqueeze(3).broadcast_to([128, nb, N, N])
                    bap = b.unsqueeze(2).broadcast_to([128, nb, N, N])
                    eng(mul_eng[f]).tensor_tensor(mm, aap, bap, mybir.AluOpType.mult)
                    off = off0 + NPAIR * f
                    for i in range(N - 1):
                        w = N - 1 - i
                        u = mv[:, :, 17 * i + 1 : 17 * i + 1 + w]
                        lo = mv[:, :, 17 * i + 16 : 17 * i + 16 + 16 * (w - 1) + 1 : 16]
                        o = ov[:, :, off : off + w]
                        eng(sub_eng[f]).tensor_tensor(
                            o, u, lo, mybir.AluOpType.subtract
                        )
                        off += w
                nc.sync.dma_start(out=outv[:, t * nb : (t + 1) * nb, :], in_=ot[:])
    nc.compile()
    return nc


_NC_CACHE = {}


def get_nc():
    cfg = os.environ.get("LBN_CFG", "")
    if cfg not in _NC_CACHE:
        kw = {}
        if cfg:
            # e.g. LBN_CFG="nb=16;mul=vvg;sub=vvg;obufs=3;mbufs=2"
            for part in cfg.split(";"):
                k, _, v = part.partition("=")
                if k in ("nb", "obufs", "mbufs"):
                    kw[k] = int(v)
                elif k == "mul":
                    kw["mul_eng"] = tuple(v)
                elif k == "sub":
                    kw["sub_eng"] = tuple(v)
        _NC_CACHE[cfg] = build_kernel(**kw)
    return _NC_CACHE[cfg]


_RUNNER = None


def _get_runner():
    """Cached jitted shard_map executable over the 8 cores (one compile)."""
    global _RUNNER
    if _RUNNER is not None:
        return _RUNNER
    import jax
    from jax.experimental.shard_map import shard_map
    from jax.sharding import Mesh, PartitionSpec

    from concourse import bass2jax

    nc = get_nc()
    bass2jax.install_neuronx_cc_hook()
    part_name = nc.partition_id_tensor.name if nc.partition_id_tensor else None
    in_names, out_names, out_avals, zero_outs = [], [], [], []
    for alloc in nc.m.functions[0].allocations:
        if not isinstance(alloc, mybir.MemoryLocationSet):
            continue
        name = alloc.memorylocations[0].name
        if alloc.kind == "ExternalInput":
            if name != part_name:
                in_names.append(name)
        elif alloc.kind == "ExternalOutput":
            shape = tuple(alloc.tensor_shape)
            dtype = mybir.dt.np(alloc.dtype)
            out_names.append(name)
            out_avals.append(jax.core.ShapedArray(shape, dtype))
            zero_outs.append(np.zeros(shape, dtype))
    all_names = in_names + out_names + ([part_name] if part_name else [])

    def _body(*args):
        operands = list(args)
        if part_name is not None:
            operands.append(bass2jax.partition_id_tensor())
        return tuple(
            bass2jax._bass_exec_p.bind(
                *operands,
                out_avals=tuple(out_avals),
                in_names=tuple(all_names),
                out_names=tuple(out_names),
                lowering_input_output_aliases=(),
                sim_require_finite=True,
                sim_require_nnan=True,
                nc=nc,
            )
        )

    devices = jax.devices()[:NCORES]
    mesh = Mesh(np.array(devices), ("core",))
    specs = (PartitionSpec("core"),) * (len(in_names) + len(out_names))
    out_specs = (PartitionSpec("core"),) * len(out_names)
    sharded = jax.jit(
        shard_map(
            _body, mesh=mesh, in_specs=specs, out_specs=out_specs, check_rep=False
        ),
        keep_unused=True,
    )
    concat_zeros = [
        np.zeros((NCORES * z.shape[0], *z.shape[1:]), z.dtype) for z in zero_outs
    ]
    _RUNNER = (sharded, in_names, concat_zeros)
    return _RUNNER


def kernel(E, px, py, pz):
    arrs = {
        "E": np.ascontiguousarray(np.asarray(E, dtype=np.float32)),
        "px": np.ascontiguousarray(np.asarray(px, dtype=np.float32)),
        "py": np.ascontiguousarray(np.asarray(py, dtype=np.float32)),
        "pz": np.ascontiguousarray(np.asarray(pz, dtype=np.float32)),
    }
    try:
        sharded, in_names, concat_zeros = _get_runner()
        outs = sharded(*[arrs[n] for n in in_names], *concat_zeros)
        return np.asarray(outs[0])
    except Exception:
        # robust fallback: the reference SPMD runner path
        nc = get_nc()
        in_maps = []
        for c in range(NCORES):
            sl = slice(c * ROWS_PER_CORE, (c + 1) * ROWS_PER_CORE)
            in_maps.append({k: v[sl] for k, v in arrs.items()})
        res = bass_utils.run_bass_kernel_spmd(
            nc, in_maps, core_ids=list(range(NCORES))
        )
        return np.concatenate([r["out"] for r in res.results], axis=0)


if __name__ == "__main__":
    rng = np.random.default_rng(0)
    ins = {
        k: rng.standard_normal((TOTAL_ROWS, N), dtype=np.float32)
        for k in ["E", "px", "py", "pz"]
    }
    out = kernel(**ins)
    print("out", out.shape, out.dtype)



# revision 3
# speedup vs baseline: 6.2013x; 6.2013x over previous
"""LBN layer kernel for Trainium2 (8 NeuronCores, pure data parallel).

Inputs: E, px, py, pz each [262144, 16] f32.
Output: [262144, 424] = concat(E, px, py, pz, cross_z, cross_x, cross_y)
where cross(a,b)[r, (i,j)] = a[r,i]*b[r,j] - a[r,j]*b[r,i] over the 120
upper-triangle pairs (i<j), for (a,b) in [(px,py),(py,pz),(pz,px)].

Per-core scheme (32768 rows/core), per tile (nb row-blocks per partition):
- 4 input DMAs (issued from the Activation engine's DGE queue) land packed
  in a small staging tile (contiguous dst, cheap descriptors).
- Act copies staging -> out-tile cols 0:64.
- DVE: 3 outer-product mults into one M3 tile [128, nb, 3, 256].
- Pool: 15 merged subtracts (one 4D op per triu diagonal, covering all 3
  features at once) into out-tile cols 64:424.
- SP issues the 27KB/partition contiguous store.
Loads and stores use separate DGE queues so stores don't head-block loads;
staging decouples compute from the out-buffer recycle chain.
"""

import os

import numpy as np

import concourse.bass as bass
import concourse.tile as tile
from concourse import bacc, bass_utils, mybir

F32 = mybir.dt.float32
N = 16
NPAIR = (N * (N - 1)) // 2  # 120
OUTW = 4 * N + 3 * NPAIR  # 424
TOTAL_ROWS = 262144
NCORES = 8
ROWS_PER_CORE = TOTAL_ROWS // NCORES  # 32768

FEAT = [("px", "py"), ("py", "pz"), ("pz", "px")]
KORD = ["E", "px", "py", "pz"]


def build_kernel(
    nrows=ROWS_PER_CORE,
    nb=16,
    sub_split=0,      # triu diagonals i < sub_split go to DVE, rest to Pool
    obufs=3,
    mbufs=2,
    sbufs=2,
    ldq="a",          # engine issuing input-load DMAs: a=Act, s=SP, v=DVE
    osplit=1,         # split the output store into this many chunks along b
    reps=1,           # repeat the whole computation in-NEFF (timing use)
):
    """Build the per-core Bass module. nb = row-blocks per partition per tile."""
    stripe = nrows // 128
    assert stripe % nb == 0
    nt = stripe // nb

    nc = bacc.Bacc("TRN2", target_bir_lowering=False, debug=False)
    ins = {
        k: nc.dram_tensor(k, [nrows, N], F32, kind="ExternalInput").ap()
        for k in KORD
    }
    out = nc.dram_tensor("out", [nrows, OUTW], F32, kind="ExternalOutput").ap()
    inv = {k: v.rearrange("(p s) f -> p s f", p=128) for k, v in ins.items()}
    outv = out.rearrange("(p s) f -> p s f", p=128)
    ldeng = {"a": nc.scalar, "s": nc.sync, "v": nc.vector}[ldq]

    with tile.TileContext(nc) as tc:
        with (
            tc.tile_pool(name="st", bufs=sbufs) as spool,
            tc.tile_pool(name="ot", bufs=obufs) as opool,
            tc.tile_pool(name="mt", bufs=mbufs) as mpool,
        ):
            for t in range(nt * reps):
                t = t % nt
                # packed staging: [128, 4, nb, 16]
                st = spool.tile([128, 4 * nb * N], F32)
                sv = st[:].rearrange("p (k b f) -> p k b f", k=4, b=nb)
                for ki, k in enumerate(KORD):
                    ldeng.dma_start(
                        out=sv[:, ki],
                        in_=inv[k][:, t * nb : (t + 1) * nb, :],
                    )
                ot = opool.tile([128, OUTW * nb], F32)
                ov = ot[:].rearrange("p (b c) -> p b c", c=OUTW)
                # Act: staging -> out cols 0:64 (dst dims mirror src k,b,f)
                od = ov[:, :, 0:64].rearrange("p b (k f) -> p k b f", k=4)
                nc.scalar.copy(out=od, in_=sv)

                # single M tile holding all 3 outer products: [p, b, 3, 256]
                mt = mpool.tile([128, nb * 3 * N * N], F32)
                mv = mt[:].rearrange("p (b f c) -> p b f c", f=3, c=N * N)
                mm = mv.rearrange("p b f (i j) -> p b f i j", j=N)
                for f, (an, bn) in enumerate(FEAT):
                    a = sv[:, KORD.index(an)]
                    b = sv[:, KORD.index(bn)]
                    aap = a.unsqueeze(3).broadcast_to([128, nb, N, N])
                    bap = b.unsqueeze(2).broadcast_to([128, nb, N, N])
                    nc.vector.tensor_tensor(
                        mm[:, :, f], aap, bap, mybir.AluOpType.mult
                    )
                # merged subtracts: one op per triu diagonal covers all 3
                # features; cross(i,j) = M[i,j] - M[j,i] at packed positions
                ovp = ov[:, :, 4 * N :].rearrange("p b (f q) -> p b f q", f=3)
                off = 0
                for i in range(N - 1):
                    w = N - 1 - i
                    u = mv[:, :, :, 17 * i + 1 : 17 * i + 1 + w]
                    lo = mv[:, :, :, 17 * i + 16 : 17 * i + 16 + 16 * (w - 1) + 1 : 16]
                    o = ovp[:, :, :, off : off + w]
                    e = nc.vector if i < sub_split else nc.gpsimd
                    e.tensor_tensor(o, u, lo, mybir.AluOpType.subtract)
                    off += w
                bc = nb // osplit
                for c in range(osplit):
                    nc.sync.dma_start(
                        out=outv[:, t * nb + c * bc : t * nb + (c + 1) * bc, :],
                        in_=ov[:, c * bc : (c + 1) * bc, :],
                    )
    nc.compile()
    return nc


_NC_CACHE = {}


def get_nc():
    cfg = os.environ.get("LBN_CFG", "")
    if cfg not in _NC_CACHE:
        kw = {}
        if cfg:
            # e.g. LBN_CFG="nb=16;sub_split=0;obufs=3;mbufs=2;sbufs=2;ldq=a"
            for part in cfg.split(";"):
                k, _, v = part.partition("=")
                if k in ("nb", "obufs", "mbufs", "sbufs", "sub_split", "osplit"):
                    kw[k] = int(v)
                elif k == "ldq":
                    kw["ldq"] = v
        _NC_CACHE[cfg] = build_kernel(**kw)
    return _NC_CACHE[cfg]


_RUNNER = None


def _get_runner():
    """Cached jitted shard_map executable over the 8 cores (one compile)."""
    global _RUNNER
    if _RUNNER is not None:
        return _RUNNER
    import jax
    from jax.experimental.shard_map import shard_map
    from jax.sharding import Mesh, PartitionSpec

    from concourse import bass2jax

    nc = get_nc()
    bass2jax.install_neuronx_cc_hook()
    part_name = nc.partition_id_tensor.name if nc.partition_id_tensor else None
    in_names, out_names, out_avals, zero_outs = [], [], [], []
    for alloc in nc.m.functions[0].allocations:
        if not isinstance(alloc, mybir.MemoryLocationSet):
            continue
        name = alloc.memorylocations[0].name
        if alloc.kind == "ExternalInput":
            if name != part_name:
                in_names.append(name)
        elif alloc.kind == "ExternalOutput":
            shape = tuple(alloc.tensor_shape)
            dtype = mybir.dt.np(alloc.dtype)
            out_names.append(name)
            out_avals.append(jax.core.ShapedArray(shape, dtype))
            zero_outs.append(np.zeros(shape, dtype))
    all_names = in_names + out_names + ([part_name] if part_name else [])

    def _body(*args):
        operands = list(args)
        if part_name is not None:
            operands.append(bass2jax.partition_id_tensor())
        return tuple(
            bass2jax._bass_exec_p.bind(
                *operands,
                out_avals=tuple(out_avals),
                in_names=tuple(all_names),
                out_names=tuple(out_names),
                lowering_input_output_aliases=(),
                sim_require_finite=True,
                sim_require_nnan=True,
                nc=nc,
            )
        )

    devices = jax.devices()[:NCORES]
    mesh = Mesh(np.array(devices), ("core",))
    specs = (PartitionSpec("core"),) * (len(in_names) + len(out_names))
    out_specs = (PartitionSpec("core"),) * len(out_names)
    sharded = jax.jit(
        shard_map(
            _body, mesh=mesh, in_specs=specs, out_specs=out_specs, check_rep=False
        ),
        keep_unused=True,
    )
    concat_zeros = [
        np.zeros((NCORES * z.shape[0], *z.shape[1:]), z.dtype) for z in zero_outs
    ]
    _RUNNER = (sharded, in_names, concat_zeros)
    return _RUNNER


def kernel(E, px, py, pz):
    arrs = {
        "E": np.ascontiguousarray(np.asarray(E, dtype=np.float32)),
        "px": np.ascontiguousarray(np.asarray(px, dtype=np.float32)),
        "py": np.ascontiguousarray(np.asarray(py, dtype=np.float32)),
        "pz": np.ascontiguousarray(np.asarray(pz, dtype=np.float32)),
    }
    try:
        sharded, in_names, concat_zeros = _get_runner()
        outs = sharded(*[arrs[n] for n in in_names], *concat_zeros)
        return np.asarray(outs[0])
    except Exception:
        # robust fallback: the reference SPMD runner path
        nc = get_nc()
        in_maps = []
        for c in range(NCORES):
            sl = slice(c * ROWS_PER_CORE, (c + 1) * ROWS_PER_CORE)
            in_maps.append({k: v[sl] for k, v in arrs.items()})
        res = bass_utils.run_bass_kernel_spmd(
            nc, in_maps, core_ids=list(range(NCORES))
        )
        return np.concatenate([r["out"] for r in res.results], axis=0)


if __name__ == "__main__":
    rng = np.random.default_rng(0)
    ins = {
        k: rng.standard_normal((TOTAL_ROWS, N), dtype=np.float32)
        for k in ["E", "px", "py", "pz"]
    }
    out = kernel(**ins)
    print("out", out.shape, out.dtype)
